# revision 6
# baseline (speedup 1.0000x reference)
"""Fused cross-attention kernel for Trainium2, data-parallel over batch on 8 cores.

Uses the low-rank structure of cross-attention (L=77 << D=512). The small
text-side factors are folded on the host:
  tn  = LayerNorm(text)                        (exact, incl. beta)
  W2  = [Wq_h @ (mask*K)_h^T]_h   [512, 308]   (K = tn @ Wk)
  W3  = [(mask*V)_h @ Wo_h]_h     [308, 512]   (V = tn @ Wv)
so per core (one batch element) the device only runs, per 128-query tile:
  S    = X @ W2                [128, 308]      (4 accumulating MMs)
  E    = exp(S * scale)                        (no max-sub: |S*scale| < 3)
  A    = E / (rowsum_h(E) - nmask_h)           (masked-softmax via count fix)
  A^T  = PE-transpose in 3 chunks of 128       (chunk 2 only 52 wide)
  out  = sum_c A^T_c.T @ W3_c  [128, 512]      (3 accumulating MMs)
"""

import sys

sys.path.insert(0, "/opt/trn_rl_repo")

import numpy as np
import ml_dtypes

import concourse.bass as bass
import concourse.mybir as mybir
import concourse.tile as tile
from concourse import bacc
from concourse.bass_utils import run_bass_kernel_spmd
from concourse.masks import make_identity

N_CORES = 8
B, T, S_, D, L, H = 8, 64, 196, 512, 77, 4
DH = D // H  # 128
NQ = T * S_  # 12544
LN_EPS = 1e-6
SCALE = float(DH) ** -0.5
P = 128
NCH = D // P  # 4 chunks of the feature dim
HL = H * L  # 308
NCC = (HL + P - 1) // P  # 3 chunks of the (head, token) dim

F32 = mybir.dt.float32
BF16 = mybir.dt.bfloat16

LAST_RESULTS = None  # BassKernelResults of the most recent run (for test harness)
_PROGRAM_CACHE = {}


def build_program(nq=NQ):
    """One SPMD program; all 8 cores run it on their own batch element."""
    nc = bacc.Bacc("TRN2", target_bir_lowering=False, debug=False, num_devices=N_CORES)

    xt = nc.dram_tensor("xt", [D, nq], BF16, kind="ExternalInput").ap()
    w2 = nc.dram_tensor("w2", [D, HL], BF16, kind="ExternalInput").ap()
    w3 = nc.dram_tensor("w3", [NCC * P, D], BF16, kind="ExternalInput").ap()
    negcnt = nc.dram_tensor("negcnt", [P, 1], F32, kind="ExternalInput").ap()
    out = nc.dram_tensor("out", [nq, D], BF16, kind="ExternalOutput").ap()

    ntiles = nq // P  # 98
    # Ramp group sizes: tiny first groups so the first matmul starts ~2us in
    # (a single big first group stalls PE ~15us waiting on a 900KB DMA).
    groups = []
    t0 = 0
    for gt in (1, 1, 2, 3):
        groups.append((t0, gt))
        t0 += gt
    GT = 7
    while t0 < ntiles:
        gt = min(GT, ntiles - t0)
        groups.append((t0, gt))
        t0 += gt

    with tile.TileContext(nc) as tc:
        with (
            tc.tile_pool(name="const", bufs=1) as const,
            tc.tile_pool(name="xtp", bufs=2) as xtp,
            tc.tile_pool(name="attp", bufs=4) as attp,
            tc.tile_pool(name="smalls", bufs=24) as smalls,
            tc.tile_pool(name="outp", bufs=4) as outp,
            tc.tile_pool(name="ps_sc", bufs=3, space="PSUM") as ps_sc,
            tc.tile_pool(name="ps_at", bufs=2, space="PSUM") as ps_at,
            tc.tile_pool(name="ps_out", bufs=3, space="PSUM") as ps_out,
        ):
            # ---- constants ----
            ident = const.tile([P, P], BF16)
            make_identity(nc, ident)

            # Prolog loads go out on distinct engine queues so they overlap.
            w2_sb = const.tile([P, NCH, HL], BF16, tag="w2")
            nc.gpsimd.dma_start(out=w2_sb[:], in_=w2.rearrange("(c p) n -> p c n", p=P))
            w3_sb = const.tile([P, NCC, D], BF16, tag="w3")
            nc.scalar.dma_start(out=w3_sb[:], in_=w3.rearrange("(c p) n -> p c n", p=P))
            negcnt_sb = const.tile([P, 1], F32, tag="negcnt")
            nc.scalar.dma_start(out=negcnt_sb[:], in_=negcnt)

            # ---- main loop ----
            for t0, gt in groups:
                qg = gt * P
                q0 = t0 * P

                xt_sb = xtp.tile([P, NCH, qg], BF16, tag="xt")
                nc.sync.dma_start(
                    out=xt_sb[:],
                    in_=xt.rearrange("(c p) q -> p c q", p=P)[:, :, q0 : q0 + qg],
                )

                for t in range(gt):
                    tq = slice(t * P, (t + 1) * P)
                    ps_s = ps_sc.tile([P, HL], F32, tag="ps_s")
                    for kc in range(NCH):
                        nc.tensor.matmul(
                            ps_s[:],
                            xt_sb[:, kc, tq],
                            w2_sb[:, kc, :],
                            start=(kc == 0),
                            stop=(kc == NCH - 1),
                        )
                    exp_sb = attp.tile([P, HL], BF16, tag="exp")
                    nc.scalar.activation(
                        exp_sb[:], ps_s[:], mybir.ActivationFunctionType.Exp,
                        scale=SCALE,
                    )
                    sumexp = smalls.tile([P, H], F32, tag="sumexp")
                    nc.vector.reduce_sum(
                        out=sumexp[:],
                        in_=exp_sb[:].rearrange("p (h l) -> p h l", h=H),
                        axis=mybir.AxisListType.X,
                    )
                    sumadj = smalls.tile([P, H], F32, tag="sumadj")
                    nc.vector.tensor_scalar_add(sumadj[:], sumexp[:], negcnt_sb[:])
                    recip = smalls.tile([P, H], F32, tag="recip")
                    nc.vector.reciprocal_approx_fast(recip[:], sumadj[:])
                    recip_b = smalls.tile([P, H], BF16, tag="recip_b")
                    nc.vector.tensor_copy(recip_b[:], recip[:])
                    attn_sb = attp.tile([P, HL], BF16, tag="attn")
                    nc.vector.tensor_mul(
                        attn_sb[:].rearrange("p (h l) -> p h l", h=H),
                        exp_sb[:].rearrange("p (h l) -> p h l", h=H),
                        recip_b[:].to_broadcast([P, H, L]),
                    )
                    # A^T in 3 partition-chunks of the 308-long (h,l) axis
                    ps_a = ps_at.tile([P, NCC * P], BF16, tag="ps_a")
                    for c in range(NCC):
                        cw = min(P, HL - c * P)
                        nc.tensor.transpose(
                            ps_a[:cw, c * P : (c + 1) * P],
                            attn_sb[:, c * P : c * P + cw],
                            ident[:],
                        )
                    attnT_sb = attp.tile([P, NCC, P], BF16, tag="attnT")
                    nc.vector.tensor_copy(
                        attnT_sb[:, 0:2, :],
                        ps_a[:, 0 : 2 * P].rearrange("p (c n) -> p c n", c=2),
                    )
                    nc.vector.tensor_copy(
                        attnT_sb[: HL - 2 * P, 2, :], ps_a[: HL - 2 * P, 2 * P :]
                    )
                    ps_o = ps_out.tile([P, D], F32, tag="ps_o")
                    for c in range(NCC):
                        cw = min(P, HL - c * P)
                        nc.tensor.matmul(
                            ps_o[:],
                            attnT_sb[:cw, c, :],
                            w3_sb[:cw, c, :],
                            start=(c == 0),
                            stop=(c == NCC - 1),
                        )
                    out_sb = outp.tile([P, D], BF16, tag="out")
                    nc.scalar.copy(out_sb[:], ps_o[:])
                    # out-DMAs ride the idle gpsimd queue so they never wait
                    # behind a multi-us xt prefetch on the sync ring
                    nc.gpsimd.dma_start(
                        out=out[q0 + t * P : q0 + (t + 1) * P, :], in_=out_sb[:]
                    )

    nc.compile()
    return nc


def _get_program(nq=NQ):
    if nq not in _PROGRAM_CACHE:
        _PROGRAM_CACHE[nq] = build_program(nq)
    return _PROGRAM_CACHE[nq]


def prep_core_inputs(visual_feat, text_feat, token_mask, wq, wk, wv, wo,
                     ln_gamma, ln_beta):
    """Host-side prep: shard over batch, fold the text side into W2/W3."""
    vf = np.ascontiguousarray(visual_feat.reshape(B, -1, D))

    # Exact LayerNorm (f32, biased variance, incl. beta)
    mu = text_feat.mean(-1, keepdims=True)
    var = np.square(text_feat - mu).mean(-1, keepdims=True)
    tn = (text_feat - mu) / np.sqrt(var + LN_EPS) * ln_gamma + ln_beta  # [B, L, D]

    m = token_mask.astype(np.float32)  # [B, L]
    k = (tn @ wk) * m[:, :, None]  # [B, L, D]
    v = (tn @ wv) * m[:, :, None]

    k4 = k.reshape(B, L, H, DH)
    v4 = v.reshape(B, L, H, DH)
    wq4 = wq.reshape(D, H, DH)
    wo4 = wo.reshape(H, DH, D)
    # W2[b, d, h*L+l] = sum_e Wq[d, (h,e)] K[b, l, (h,e)]
    w2_all = np.einsum("dhe,blhe->bdhl", wq4, k4, optimize=True).reshape(B, D, HL)
    # W3[b, h*L+l, d] = sum_e V[b, l, (h,e)] Wo[(h,e), d]
    w3_all = np.einsum("blhe,hed->bhld", v4, wo4, optimize=True).reshape(B, HL, D)
    w3_pad = np.zeros((B, NCC * P, D), np.float32)
    w3_pad[:, :HL, :] = w3_all

    in_maps = []
    for b in range(B):
        xt = np.ascontiguousarray(vf[b].T).astype(ml_dtypes.bfloat16)
        negcnt = np.full((P, 1), -(L - float(m[b].sum())), np.float32)
        in_maps.append({
            "xt": xt,
            "w2": w2_all[b].astype(ml_dtypes.bfloat16),
            "w3": w3_pad[b].astype(ml_dtypes.bfloat16),
            "negcnt": negcnt,
        })
    return in_maps, np.zeros((D,), np.float32)


def kernel(visual_feat, text_feat, token_mask, Wq, Wk, Wv, Wo, ln_gamma, ln_beta):
    global LAST_RESULTS
    visual_feat = np.asarray(visual_feat, np.float32)
    text_feat = np.asarray(text_feat, np.float32)
    token_mask = np.asarray(token_mask)

    in_maps, out_corr = prep_core_inputs(
        visual_feat, text_feat, token_mask,
        np.asarray(Wq, np.float32), np.asarray(Wk, np.float32),
        np.asarray(Wv, np.float32), np.asarray(Wo, np.float32),
        np.asarray(ln_gamma, np.float32), np.asarray(ln_beta, np.float32),
    )
    nc = _get_program()
    res = run_bass_kernel_spmd(nc, in_maps, core_ids=list(range(N_CORES)))
    LAST_RESULTS = res
    out = np.stack([res.results[b]["out"].astype(np.float32) for b in range(B)], axis=0)
    if np.any(out_corr):
        out = out + out_corr[None, None, :]
    return out.reshape(B, T, S_, D)


# revision 15
# speedup vs baseline: 1.1831x; 1.1831x over previous
"""Fused cross-attention kernel for Trainium2, data-parallel over batch on 8 cores.

Uses the low-rank structure of cross-attention (L=77 << D=512). The small
text-side factors are folded on the host:
  tn  = LayerNorm(text)                        (exact, incl. beta)
  W2  = [Wq_h @ (mask*K)_h^T]_h   [512, 308]   (K = tn @ Wk)
  W3  = [(mask*V)_h @ Wo_h]_h     [308, 512]   (V = tn @ Wv)
so per core (one batch element) the device only runs, per 128-query tile:
  S    = X @ W2                [128, 308]      (4 accumulating MMs)
  E    = exp(S * scale)                        (no max-sub: |S*scale| < 3)
  A    = E / (rowsum_h(E) - nmask_h)           (masked-softmax via count fix)
  A^T  = PE-transpose in 3 chunks of 128       (chunk 2 only 52 wide)
  out  = sum_c A^T_c.T @ W3_c  [128, 512]      (3 accumulating MMs)
"""

import sys

sys.path.insert(0, "/opt/trn_rl_repo")

import numpy as np
import ml_dtypes

import concourse.bass as bass
import concourse.mybir as mybir
import concourse.tile as tile
from concourse import bacc
from concourse.bass_utils import run_bass_kernel_spmd
from concourse.masks import make_identity

N_CORES = 8
B, T, S_, D, L, H = 8, 64, 196, 512, 77, 4
DH = D // H  # 128
NQ = T * S_  # 12544
LN_EPS = 1e-6
SCALE = float(DH) ** -0.5
P = 128
NCH = D // P  # 4 chunks of the feature dim
HL = H * L  # 308
NCC = (HL + P - 1) // P  # 3 chunks of the (head, token) dim

F32 = mybir.dt.float32
BF16 = mybir.dt.bfloat16
F8E4 = mybir.dt.float8e4

# fp8 (e4m3) scores path: X and W2 in fp8, scores via DoubleRow matmuls
# (2 accumulating MMs contracting 256 each instead of 4 contracting 128).
FP8_SCORES = True
W2SCALE = 8.0  # host multiplies W2 by this pre-fp8-cast; exp scale divides it out
W2PAD = 320  # w2 free dim padded so the DoubleRow k-tile stride is 16B-aligned

LAST_RESULTS = None  # BassKernelResults of the most recent run (for test harness)
_PROGRAM_CACHE = {}


def build_program(nq=NQ):
    """One SPMD program; all 8 cores run it on their own batch element."""
    nc = bacc.Bacc("TRN2", target_bir_lowering=False, debug=False, num_devices=N_CORES)

    xdt = F8E4 if FP8_SCORES else BF16
    w2w = W2PAD if FP8_SCORES else HL
    xt = nc.dram_tensor("xt", [D, nq], xdt, kind="ExternalInput").ap()
    w2 = nc.dram_tensor("w2", [D, w2w], xdt, kind="ExternalInput").ap()
    w3 = nc.dram_tensor("w3", [NCC * P, D], BF16, kind="ExternalInput").ap()
    negcnt = nc.dram_tensor("negcnt", [P, 1], F32, kind="ExternalInput").ap()
    out = nc.dram_tensor("out", [nq, D], BF16, kind="ExternalOutput").ap()

    ntiles = nq // P  # 98
    # Ramp group sizes: tiny first groups so the first matmul starts ~2us in
    # (a single big first group stalls PE ~15us waiting on a 900KB DMA).
    groups = []
    t0 = 0
    for gt in (1, 1, 2, 3):
        groups.append((t0, gt))
        t0 += gt
    GT = 7
    while t0 < ntiles:
        gt = min(GT, ntiles - t0)
        groups.append((t0, gt))
        t0 += gt

    with tile.TileContext(nc) as tc:
        with (
            tc.tile_pool(name="const", bufs=1) as const,
            tc.tile_pool(name="xtp", bufs=2) as xtp,
            tc.tile_pool(name="attp", bufs=4) as attp,
            tc.tile_pool(name="smalls", bufs=24) as smalls,
            tc.tile_pool(name="outp", bufs=4) as outp,
            tc.tile_pool(name="ps_sc", bufs=3, space="PSUM") as ps_sc,
            tc.tile_pool(name="ps_at", bufs=2, space="PSUM") as ps_at,
            tc.tile_pool(name="ps_out", bufs=3, space="PSUM") as ps_out,
        ):
            # ---- constants ----
            ident = const.tile([P, P], BF16)
            make_identity(nc, ident)

            # Prolog loads: w2 first (gates the first matmul), w3 on the
            # scalar HWDGE queue so it overlaps the sync-queue traffic.
            w2_sb = const.tile([P, NCH, w2w], xdt, tag="w2")
            nc.sync.dma_start(out=w2_sb[:], in_=w2.rearrange("(c p) n -> p c n", p=P))
            w3_sb = const.tile([P, NCC, D], BF16, tag="w3")
            nc.scalar.dma_start(out=w3_sb[:], in_=w3.rearrange("(c p) n -> p c n", p=P))
            negcnt_sb = const.tile([P, 1], F32, tag="negcnt")
            nc.scalar.dma_start(out=negcnt_sb[:], in_=negcnt)

            # ---- main loop ----
            for t0, gt in groups:
                qg = gt * P
                q0 = t0 * P

                xt_sb = xtp.tile([P, NCH, qg], xdt, tag="xt")
                nc.sync.dma_start(
                    out=xt_sb[:],
                    in_=xt.rearrange("(c p) q -> p c q", p=P)[:, :, q0 : q0 + qg],
                )

                for t in range(gt):
                    tq = slice(t * P, (t + 1) * P)
                    ps_s = ps_sc.tile([P, HL], F32, tag="ps_s")
                    if FP8_SCORES:
                        for c in range(2):
                            nc.tensor.matmul(
                                ps_s[:],
                                xt_sb[:, 2 * c : 2 * c + 2, tq],
                                w2_sb[:, 2 * c : 2 * c + 2, :HL],
                                start=(c == 0),
                                stop=(c == 1),
                                perf_mode=mybir.MatmulPerfMode.DoubleRow,
                            )
                    else:
                        for kc in range(NCH):
                            nc.tensor.matmul(
                                ps_s[:],
                                xt_sb[:, kc, tq],
                                w2_sb[:, kc, :],
                                start=(kc == 0),
                                stop=(kc == NCH - 1),
                            )
                    exp_sb = attp.tile([P, HL], BF16, tag="exp")
                    nc.scalar.activation(
                        exp_sb[:], ps_s[:], mybir.ActivationFunctionType.Exp,
                        scale=SCALE / W2SCALE if FP8_SCORES else SCALE,
                    )
                    sumexp = smalls.tile([P, H], F32, tag="sumexp")
                    nc.vector.reduce_sum(
                        out=sumexp[:],
                        in_=exp_sb[:].rearrange("p (h l) -> p h l", h=H),
                        axis=mybir.AxisListType.X,
                    )
                    # small softmax chain split DVE<->gpsimd to keep the
                    # vector engine under the PE's per-tile budget
                    sumadj = smalls.tile([P, H], F32, tag="sumadj")
                    nc.gpsimd.tensor_scalar_add(sumadj[:], sumexp[:], negcnt_sb[:])
                    recip = smalls.tile([P, H], F32, tag="recip")
                    nc.vector.reciprocal_approx_fast(recip[:], sumadj[:])
                    recip_b = smalls.tile([P, H], BF16, tag="recip_b")
                    nc.gpsimd.tensor_copy(recip_b[:], recip[:])
                    attn_sb = attp.tile([P, HL], BF16, tag="attn")
                    nc.gpsimd.tensor_mul(
                        attn_sb[:].rearrange("p (h l) -> p h l", h=H),
                        exp_sb[:].rearrange("p (h l) -> p h l", h=H),
                        recip_b[:].to_broadcast([P, H, L]),
                    )
                    # A^T in 3 partition-chunks of the 308-long (h,l) axis
                    ps_a = ps_at.tile([P, NCC * P], BF16, tag="ps_a")
                    for c in range(NCC):
                        cw = min(P, HL - c * P)
                        nc.tensor.transpose(
                            ps_a[:cw, c * P : (c + 1) * P],
                            attn_sb[:, c * P : c * P + cw],
                            ident[:],
                        )
                    attnT_sb = attp.tile([P, NCC, P], BF16, tag="attnT")
                    nc.vector.tensor_copy(
                        attnT_sb[:, 0:2, :],
                        ps_a[:, 0 : 2 * P].rearrange("p (c n) -> p c n", c=2),
                    )
                    nc.vector.tensor_copy(
                        attnT_sb[: HL - 2 * P, 2, :], ps_a[: HL - 2 * P, 2 * P :]
                    )
                    ps_o = ps_out.tile([P, D], F32, tag="ps_o")
                    for c in range(NCC):
                        cw = min(P, HL - c * P)
                        nc.tensor.matmul(
                            ps_o[:],
                            attnT_sb[:cw, c, :],
                            w3_sb[:cw, c, :],
                            start=(c == 0),
                            stop=(c == NCC - 1),
                        )
                    out_sb = outp.tile([P, D], BF16, tag="out")
                    nc.scalar.copy(out_sb[:], ps_o[:])
                    nc.sync.dma_start(
                        out=out[q0 + t * P : q0 + (t + 1) * P, :], in_=out_sb[:]
                    )

    nc.compile()
    return nc


def _get_program(nq=NQ):
    if nq not in _PROGRAM_CACHE:
        _PROGRAM_CACHE[nq] = build_program(nq)
    return _PROGRAM_CACHE[nq]


def prep_core_inputs(visual_feat, text_feat, token_mask, wq, wk, wv, wo,
                     ln_gamma, ln_beta):
    """Host-side prep: shard over batch, fold the text side into W2/W3."""
    vf = np.ascontiguousarray(visual_feat.reshape(B, -1, D))

    # Exact LayerNorm (f32, biased variance, incl. beta)
    mu = text_feat.mean(-1, keepdims=True)
    var = np.square(text_feat - mu).mean(-1, keepdims=True)
    tn = (text_feat - mu) / np.sqrt(var + LN_EPS) * ln_gamma + ln_beta  # [B, L, D]

    m = token_mask.astype(np.float32)  # [B, L]
    k = (tn @ wk) * m[:, :, None]  # [B, L, D]
    v = (tn @ wv) * m[:, :, None]

    k4 = k.reshape(B, L, H, DH)
    v4 = v.reshape(B, L, H, DH)
    wq4 = wq.reshape(D, H, DH)
    wo4 = wo.reshape(H, DH, D)
    # W2[b, d, h*L+l] = sum_e Wq[d, (h,e)] K[b, l, (h,e)]
    w2_all = np.einsum("dhe,blhe->bdhl", wq4, k4, optimize=True).reshape(B, D, HL)
    # W3[b, h*L+l, d] = sum_e V[b, l, (h,e)] Wo[(h,e), d]
    w3_all = np.einsum("blhe,hed->bhld", v4, wo4, optimize=True).reshape(B, HL, D)
    w3_pad = np.zeros((B, NCC * P, D), np.float32)
    w3_pad[:, :HL, :] = w3_all

    in_maps = []
    for b in range(B):
        xt = np.ascontiguousarray(vf[b].T)
        if FP8_SCORES:
            # TRN FP8_EXP4 is e4m3 with max +-240 (256+ decodes as inf/nan)
            xt_c = np.clip(xt, -240, 240).astype(ml_dtypes.float8_e4m3fn)
            w2_c = np.zeros((D, W2PAD), np.float32)
            w2_c[:, :HL] = w2_all[b] * W2SCALE
            w2_c = np.clip(w2_c, -240, 240).astype(ml_dtypes.float8_e4m3fn)
        else:
            xt_c = xt.astype(ml_dtypes.bfloat16)
            w2_c = w2_all[b].astype(ml_dtypes.bfloat16)
        negcnt = np.full((P, 1), -(L - float(m[b].sum())), np.float32)
        in_maps.append({
            "xt": xt_c,
            "w2": w2_c,
            "w3": w3_pad[b].astype(ml_dtypes.bfloat16),
            "negcnt": negcnt,
        })
    return in_maps, np.zeros((D,), np.float32)


def kernel(visual_feat, text_feat, token_mask, Wq, Wk, Wv, Wo, ln_gamma, ln_beta):
    global LAST_RESULTS
    visual_feat = np.asarray(visual_feat, np.float32)
    text_feat = np.asarray(text_feat, np.float32)
    token_mask = np.asarray(token_mask)

    in_maps, out_corr = prep_core_inputs(
        visual_feat, text_feat, token_mask,
        np.asarray(Wq, np.float32), np.asarray(Wk, np.float32),
        np.asarray(Wv, np.float32), np.asarray(Wo, np.float32),
        np.asarray(ln_gamma, np.float32), np.asarray(ln_beta, np.float32),
    )
    nc = _get_program()
    res = run_bass_kernel_spmd(nc, in_maps, core_ids=list(range(N_CORES)))
    LAST_RESULTS = res
    out = np.stack([res.results[b]["out"].astype(np.float32) for b in range(B)], axis=0)
    if np.any(out_corr):
        out = out + out_corr[None, None, :]
    return out.reshape(B, T, S_, D)
